# revision 25
# baseline (speedup 1.0000x reference)
"""SATD-style custom loss on 8 Trainium2 NeuronCores.

Computes sum(|H8 @ (original - pred)|) where H8 is the 8x8 Sylvester
Hadamard matrix applied along dim -2 of [B, C, 8, 8] blocks.

Strategy: pure data parallel over the block-batch dim (8 shards).
Per core:
  - gpsimd DMA loads with inline fp32->bf16 cast (halves on-chip traffic)
  - DVE: diff + 3-stage fast Walsh-Hadamard transform along j
    (butterfly distances 8/16/32 elements within each 64-elem block)
  - ACT: fused Abs + per-partition accumulate (accum_out)
  - final DVE reduce to [128,1] per core; host sums 8x128 partials.
"""

import numpy as np

from operator import add as _op_add

import concourse.bacc as bacc
import concourse.bass as bass
import concourse.mybir as mybir
from concourse import dve_ops
from concourse.bass_utils import run_bass_kernel_spmd
from concourse.dve_spec import Spec, Src0, Src1, Zero, lower, maxx
from concourse.dve_uop import DveOpSpec
from concourse.tile import TileContext

# Problem shape (hardcoded; kernel.py must be self-contained).
N_BLOCKS = 524288
C = 3
N_CORES = 8
ELEMS_PER_CORE = (N_BLOCKS // N_CORES) * C * 64  # 12_582_912
P = 128  # SBUF partitions
F = 4096  # fp32 elems per partition per tile
ROWS = ELEMS_PER_CORE // F  # 3072
T = ROWS // P  # 24 tiles per core
NB = F // 64  # 64 SATD blocks per partition per tile

CAST_ON_DMA = True  # fp32->bf16 during DMA (SWDGE); else cast in the diff op
# Full diff on DVE: GPSIMD compute blocks its own SWDGE DMA descriptor
# generation (measured +60us when offloading diff columns to GPSIMD).
DVE_DIFF = 4096


def _register_absmax_sum_op() -> "dve_ops.DveOp":
    """Custom fused DVE op: out = max(|in0|,|in1|); accum_out = sum(out).

    Used to collapse the last FWHT butterfly: |e+o| + |e-o| == 2*max(|e|,|o|),
    so one op replaces the stage-3 add/sub pair plus the abs+accumulate pass.
    """
    name = "ABS_MAX_SUM_SATD"
    for op in dve_ops.OPS:
        if op.name == name:
            return op
    import numpy as np_

    spec = Spec(
        body=maxx(maxx(Src0, Zero - Src0), maxx(Src1, Zero - Src1)),
        accum=_op_add,
        reference=lambda in0, in1, s0, s1, imm2: np_.maximum(
            np_.abs(in0), np_.abs(in1)
        ),
    )
    row = max(dve_ops._SUB_OPCODE_FOR_NAME.values()) + 1
    dve_ops._SUB_OPCODE_FOR_NAME[name] = row
    ver = "v3"  # TRN2
    sha = DveOpSpec(
        name=name, opcode=row, uops=lower(spec, ver=ver), rd1_en=True
    ).sha(ver)
    # perf_en off: the 2-port DVE perf mode locks GPSIMD out of SBUF and
    # stalls SWDGE DMA descriptor generation (measured +38us in drains).
    op = dve_ops.DveOp(
        name, spec, subdim=False, uops_sha={ver: sha}, perf_en={ver: False}
    )
    dve_ops.OPS.append(op)
    dve_ops.CUSTOM_DVE_SPECS[name] = spec
    return op


_ABSMAX_OP = _register_absmax_sum_op()


def _build_program() -> bass.Bass:
    nc = bacc.Bacc("TRN2", debug=False, num_devices=N_CORES)
    dt = mybir.dt

    # Host interleaves o|p per row: x[r] = [o_row_r (F), p_row_r (F)].
    # One DMA per tile -> in-order tile completion, single dep for the diff.
    x_dram = nc.declare_dram_parameter("x", [ROWS, 2 * F], dt.float32, isOutput=False)
    out_dram = nc.declare_dram_parameter("out", [P, 1], dt.float32, isOutput=True)

    in_dt = dt.bfloat16 if CAST_ON_DMA else dt.float32

    with TileContext(nc) as tc:
        with (
            tc.tile_pool(name="io", bufs=4) as io_pool,
            tc.tile_pool(name="work", bufs=3) as work_pool,
            tc.tile_pool(name="acc", bufs=1) as acc_pool,
        ):
            acc = acc_pool.tile([P, T], dt.float32)

            for t in range(T):
                dma_eng = nc.gpsimd if CAST_ON_DMA else nc.sync
                xb = io_pool.tile([P, 2 * F], in_dt, tag="xb")
                dma_eng.dma_start(out=xb[:], in_=x_dram[t * P : (t + 1) * P, :])

                # diff of the o-half and p-half; split DVE/GPSIMD to offload
                # the busiest engine (DVE). GPSIMD 2-input floor is ~4x DVE
                # bf16 rate, so it gets a proportionally smaller slice.
                d0 = work_pool.tile([P, F], dt.bfloat16, tag="d0")
                nc.vector.tensor_sub(
                    d0[:, 0:DVE_DIFF], xb[:, 0:DVE_DIFF], xb[:, F : F + DVE_DIFF]
                )
                if DVE_DIFF < F:
                    nc.gpsimd.tensor_sub(
                        d0[:, DVE_DIFF:F],
                        xb[:, DVE_DIFF:F],
                        xb[:, F + DVE_DIFF : 2 * F],
                    )

                # FWHT along j: free offset within a block = j*8 + w.
                # stage 1: combine j-bit0 (element distance 8)
                d1 = work_pool.tile([P, F], dt.bfloat16, tag="d1")
                v0 = d0[:].rearrange("p (b j2 s w) -> p b j2 s w", j2=4, s=2, w=8)
                v1 = d1[:].rearrange("p (b j2 s w) -> p b j2 s w", j2=4, s=2, w=8)
                nc.vector.tensor_add(v1[:, :, :, 0, :], v0[:, :, :, 0, :], v0[:, :, :, 1, :])
                nc.vector.tensor_sub(v1[:, :, :, 1, :], v0[:, :, :, 0, :], v0[:, :, :, 1, :])

                # stage 2: combine j-bit1 (element distance 16)
                d2 = work_pool.tile([P, F], dt.bfloat16, tag="d2")
                w1 = d1[:].rearrange("p (b jh s jl) -> p b jh s jl", jh=2, s=2, jl=16)
                w2 = d2[:].rearrange("p (b jh s jl) -> p b jh s jl", jh=2, s=2, jl=16)
                nc.vector.tensor_add(w2[:, :, :, 0, :], w1[:, :, :, 0, :], w1[:, :, :, 1, :])
                nc.vector.tensor_sub(w2[:, :, :, 1, :], w1[:, :, :, 0, :], w1[:, :, :, 1, :])

                # stage 3 + abs, collapsed: for the final butterfly pair,
                # |e+o| + |e-o| == 2*max(|e|,|o|), so one abs_max over the
                # distance-32 pairs replaces stage 3 and the abs. The global
                # factor of 2 is applied on the host.
                # stage 3 + abs + accumulate, fully fused: for the final
                # butterfly pair, |e+o| + |e-o| == 2*max(|e|,|o|); a custom
                # DVE op computes max(|e|,|o|) and the per-partition sum in
                # one pass. The factor of 2 is applied on the host.
                m = work_pool.tile([P, F // 2], dt.bfloat16, tag="m")
                x2 = d2[:].rearrange("p (b s jl) -> p b s jl", s=2, jl=32)
                xm = m[:].rearrange("p (b jl) -> p b jl", jl=32)
                nc.vector._custom_dve(
                    _ABSMAX_OP,
                    out=xm,
                    in0=x2[:, :, 0, :],
                    in1=x2[:, :, 1, :],
                    accum_out=acc[:, t : t + 1],
                )

            accsum = acc_pool.tile([P, 1], dt.float32)
            nc.vector.tensor_reduce(
                out=accsum[:],
                in_=acc[:],
                axis=mybir.AxisListType.X,
                op=mybir.AluOpType.add,
            )
            nc.sync.dma_start(out=out_dram[:, :], in_=accsum[:])

    nc.compile()
    return nc


_NC_CACHE: bass.Bass | None = None


def _get_program() -> bass.Bass:
    global _NC_CACHE
    if _NC_CACHE is None:
        _NC_CACHE = _build_program()
    return _NC_CACHE


def run(original: np.ndarray, pred: np.ndarray, trace: bool = False, **kwargs):
    """Shard, run on 8 cores, return (scalar result, BassKernelResults)."""
    o = np.asarray(original, dtype=np.float32).reshape(N_CORES, ROWS, F)
    p = np.asarray(pred, dtype=np.float32).reshape(N_CORES, ROWS, F)
    x = np.concatenate([o, p], axis=2)  # [N_CORES, ROWS, 2F] row-interleaved
    in_maps = [{"x": x[i]} for i in range(N_CORES)]
    nc = _get_program()
    res = run_bass_kernel_spmd(
        nc, in_maps, core_ids=list(range(N_CORES)), trace=trace, **kwargs
    )
    total = np.float64(0.0)
    for r in res.results:
        total += r["out"].astype(np.float64).sum()
    # x2: the abs_max identity halves the summed element count on-device.
    return np.array(2.0 * total, dtype=np.float32), res


def kernel(original: np.ndarray, pred: np.ndarray) -> np.ndarray:
    out, _ = run(original, pred, trace=False)
    return out


# revision 33
# speedup vs baseline: 1.2289x; 1.2289x over previous
"""SATD-style custom loss on 8 Trainium2 NeuronCores.

Computes sum(|H8 @ (original - pred)|) where H8 is the 8x8 Sylvester
Hadamard matrix applied along dim -2 of [B, C, 8, 8] blocks.

Strategy: pure data parallel over the block-batch dim (8 shards).
Per core:
  - gpsimd DMA loads with inline fp32->bf16 cast (halves on-chip traffic)
  - DVE: diff + 3-stage fast Walsh-Hadamard transform along j
    (butterfly distances 8/16/32 elements within each 64-elem block)
  - ACT: fused Abs + per-partition accumulate (accum_out)
  - final DVE reduce to [128,1] per core; host sums 8x128 partials.
"""

import numpy as np

import concourse.bacc as bacc
import concourse.bass as bass
import concourse.mybir as mybir
from concourse.bass_utils import run_bass_kernel_spmd
from concourse.tile import TileContext

# Problem shape (hardcoded; kernel.py must be self-contained).
N_BLOCKS = 524288
C = 3
N_CORES = 8
ELEMS_PER_CORE = (N_BLOCKS // N_CORES) * C * 64  # 12_582_912
P = 128  # SBUF partitions
F = 4096  # fp32 elems per partition per tile
ROWS = ELEMS_PER_CORE // F  # 3072
T = ROWS // P  # 24 tiles per core
NB = F // 64  # 64 SATD blocks per partition per tile

CAST_ON_DMA = True  # fp32->bf16 during DMA (SWDGE); else cast in the diff op
def _build_program() -> bass.Bass:
    nc = bacc.Bacc("TRN2", debug=False, num_devices=N_CORES)
    dt = mybir.dt

    # Host interleaves o|p per row: x[r] = [o_row_r (F), p_row_r (F)].
    # One DMA per tile -> in-order tile completion, single dep for the diff.
    x_dram = nc.declare_dram_parameter("x", [ROWS, 2 * F], dt.float32, isOutput=False)
    out_dram = nc.declare_dram_parameter("out", [P, 1], dt.float32, isOutput=True)

    in_dt = dt.bfloat16 if CAST_ON_DMA else dt.float32

    with TileContext(nc) as tc:
        with (
            tc.tile_pool(name="io", bufs=4) as io_pool,
            tc.tile_pool(name="work", bufs=3) as work_pool,
            tc.tile_pool(name="acc", bufs=1) as acc_pool,
        ):
            # Tile plan: (row0, col0, width). First and last tiles are split
            # into quarters: small first chunks let DVE start ~7us earlier
            # (it otherwise never recovers the startup lag — DVE and DMA
            # per-tile times are nearly matched); small last chunks shorten
            # the serial drain chain at the end.
            SPLIT = 4
            w_q = F // SPLIT
            plan = (
                [(0, k * w_q, w_q) for k in range(SPLIT)]
                + [(t * P, 0, F) for t in range(1, T - 1)]
                + [((T - 1) * P, k * w_q, w_q) for k in range(SPLIT)]
            )
            acc = acc_pool.tile([P, len(plan)], dt.float32)

            for t, (r0, c0, F_) in enumerate(plan):
                dma_eng = nc.gpsimd if CAST_ON_DMA else nc.sync
                xb = io_pool.tile([P, 2 * F_], in_dt, tag="xb")
                if c0 == 0 and F_ == F:
                    dma_eng.dma_start(out=xb[:], in_=x_dram[r0 : r0 + P, :])
                else:
                    dma_eng.dma_start(
                        out=xb[:, 0:F_], in_=x_dram[r0 : r0 + P, c0 : c0 + F_]
                    )
                    dma_eng.dma_start(
                        out=xb[:, F_ : 2 * F_],
                        in_=x_dram[r0 : r0 + P, F + c0 : F + c0 + F_],
                    )

                # diff of the o-half and p-half
                d0 = work_pool.tile([P, F_], dt.bfloat16, tag="d0")
                nc.vector.tensor_sub(d0[:], xb[:, 0:F_], xb[:, F_ : 2 * F_])

                # FWHT along j: free offset within a block = j*8 + w.
                # stage 1: combine j-bit0 (element distance 8)
                d1 = work_pool.tile([P, F_], dt.bfloat16, tag="d1")
                v0 = d0[:].rearrange("p (b j2 s w) -> p b j2 s w", j2=4, s=2, w=8)
                v1 = d1[:].rearrange("p (b j2 s w) -> p b j2 s w", j2=4, s=2, w=8)
                nc.vector.tensor_add(v1[:, :, :, 0, :], v0[:, :, :, 0, :], v0[:, :, :, 1, :])
                nc.vector.tensor_sub(v1[:, :, :, 1, :], v0[:, :, :, 0, :], v0[:, :, :, 1, :])

                # stage 2: combine j-bit1 (element distance 16)
                d2 = work_pool.tile([P, F_], dt.bfloat16, tag="d2")
                w1 = d1[:].rearrange("p (b jh s jl) -> p b jh s jl", jh=2, s=2, jl=16)
                w2 = d2[:].rearrange("p (b jh s jl) -> p b jh s jl", jh=2, s=2, jl=16)
                nc.vector.tensor_add(w2[:, :, :, 0, :], w1[:, :, :, 0, :], w1[:, :, :, 1, :])
                nc.vector.tensor_sub(w2[:, :, :, 1, :], w1[:, :, :, 0, :], w1[:, :, :, 1, :])

                # stage 3 + abs, collapsed: for the final butterfly pair,
                # |e+o| + |e-o| == 2*max(|e|,|o|), so one abs_max over the
                # distance-32 pairs replaces stage 3 and the abs. The global
                # factor of 2 is applied on the host.
                # stage 3: combine j-bit2 (element distance 32)
                d3 = work_pool.tile([P, F_], dt.bfloat16, tag="d3")
                x2 = d2[:].rearrange("p (b s jl) -> p b s jl", s=2, jl=32)
                x3 = d3[:].rearrange("p (b s jl) -> p b s jl", s=2, jl=32)
                nc.vector.tensor_add(x3[:, :, 0, :], x2[:, :, 0, :], x2[:, :, 1, :])
                nc.vector.tensor_sub(x3[:, :, 1, :], x2[:, :, 0, :], x2[:, :, 1, :])

                # abs + per-partition running sum for this tile (ACT engine);
                # elementwise out is a dump into d2 (dead after stage 3).
                nc.scalar.activation(
                    out=d2[:],
                    in_=d3[:],
                    func=mybir.ActivationFunctionType.Abs,
                    accum_out=acc[:, t : t + 1],
                )

            accsum = acc_pool.tile([P, 1], dt.float32)
            nc.vector.tensor_reduce(
                out=accsum[:],
                in_=acc[:],
                axis=mybir.AxisListType.X,
                op=mybir.AluOpType.add,
            )
            nc.sync.dma_start(out=out_dram[:, :], in_=accsum[:])

    nc.compile()
    return nc


_NC_CACHE: bass.Bass | None = None


def _get_program() -> bass.Bass:
    global _NC_CACHE
    if _NC_CACHE is None:
        _NC_CACHE = _build_program()
    return _NC_CACHE


def run(original: np.ndarray, pred: np.ndarray, trace: bool = False, **kwargs):
    """Shard, run on 8 cores, return (scalar result, BassKernelResults)."""
    o = np.asarray(original, dtype=np.float32).reshape(N_CORES, ROWS, F)
    p = np.asarray(pred, dtype=np.float32).reshape(N_CORES, ROWS, F)
    x = np.concatenate([o, p], axis=2)  # [N_CORES, ROWS, 2F] row-interleaved
    in_maps = [{"x": x[i]} for i in range(N_CORES)]
    nc = _get_program()
    res = run_bass_kernel_spmd(
        nc, in_maps, core_ids=list(range(N_CORES)), trace=trace, **kwargs
    )
    total = np.float64(0.0)
    for r in res.results:
        total += r["out"].astype(np.float64).sum()
    return np.array(total, dtype=np.float32), res


def kernel(original: np.ndarray, pred: np.ndarray) -> np.ndarray:
    out, _ = run(original, pred, trace=False)
    return out
